# revision 2
# baseline (speedup 1.0000x reference)
"""Hadamard transform kernel for Trainium2 (8 NeuronCores, SPMD).

Problem: x (8192, 4096) fp32; apply a 128-point Hadamard transform to each
contiguous 128-element group of every row.  Equivalent to
    out = (x.reshape(-1, 128) @ M).reshape(8192, 4096)
where M is the 128x128 butterfly matrix (symmetric, entries +/- 2^-3.5).

Strategy per core (rows sharded 8 ways -> 1024 rows/core):
  - DMA a 128-row tile [128, 4096] to SBUF (rows on partitions).
  - For each 128-col group: PE-transpose the 128x128 block into PSUM
    (contraction dim must live on partitions), copy PSUM->SBUF,
    then matmul(lhsT=block^T, rhs=M) -> PSUM gives the transformed block
    back in natural orientation; copy PSUM->SBUF and DMA the tile out.
"""

import math

import numpy as np

import concourse.bass as bass
import concourse.tile as tile
from concourse import bacc, mybir
from concourse.bass import ts
from concourse.bass_utils import run_bass_kernel_spmd

N_CORES = 8
ROWS, COLS = 8192, 4096
R_CORE = ROWS // N_CORES  # 1024 rows per core
G = 128                   # hadamard group size
NG = COLS // G            # 32 groups per row
NT = R_CORE // 128        # 8 row-tiles per core
QUADS = NG // 4           # 4 groups (one PSUM bank) per quad


def _hadamard_matrix() -> np.ndarray:
    """M = butterfly(I_128): out_row = x_row @ M (M symmetric)."""
    x = np.eye(G, dtype=np.float64)[..., None]
    for _ in range(int(math.log2(G))):
        top = x[..., ::2, :] + x[..., 1::2, :]
        bot = x[..., ::2, :] - x[..., 1::2, :]
        x = np.concatenate((top, bot), axis=-1) * (0.5 ** 0.5)
    return np.ascontiguousarray(x.squeeze(-2).astype(np.float32))


def _build_module():
    nc = bacc.Bacc("TRN2", target_bir_lowering=False, debug=False)
    f32 = mybir.dt.float32
    x_d = nc.dram_tensor("x", [R_CORE, COLS], f32, kind="ExternalInput")
    h_d = nc.dram_tensor("hmat", [G, G], f32, kind="ExternalInput")
    i_d = nc.dram_tensor("ident", [G, G], f32, kind="ExternalInput")
    o_d = nc.dram_tensor("out", [R_CORE, COLS], f32, kind="ExternalOutput")

    with tile.TileContext(nc) as tc:
        with (
            tc.tile_pool(name="const", bufs=1) as cpool,
            tc.tile_pool(name="xin", bufs=2) as xpool,
            tc.tile_pool(name="tt", bufs=4) as tpool,
            tc.tile_pool(name="outb", bufs=2) as opool,
            tc.tile_pool(name="pst", bufs=3, space=bass.MemorySpace.PSUM) as pst,
            tc.tile_pool(name="psm", bufs=3, space=bass.MemorySpace.PSUM) as psm,
        ):
            hm = cpool.tile([G, G], f32)
            idm = cpool.tile([G, G], f32)
            nc.sync.dma_start(hm[:], h_d[:])
            nc.sync.dma_start(idm[:], i_d[:])

            for t in range(NT):
                xt = xpool.tile([128, COLS], f32)
                nc.sync.dma_start(xt[:], x_d[t * 128:(t + 1) * 128, :])
                ot = opool.tile([128, COLS], f32)
                for q in range(QUADS):
                    pt = pst.tile([128, 512], f32)
                    for j in range(4):
                        g = q * 4 + j
                        nc.tensor.transpose(
                            pt[:, ts(j, G)], xt[:, ts(g, G)], idm[:]
                        )
                    tt = tpool.tile([128, 512], f32)
                    nc.vector.tensor_copy(tt[:], pt[:])
                    pm = psm.tile([128, 512], f32)
                    for j in range(4):
                        nc.tensor.matmul(
                            pm[:, ts(j, G)], tt[:, ts(j, G)], hm[:]
                        )
                    nc.scalar.copy(ot[:, ts(q, 512)], pm[:])
                nc.sync.dma_start(o_d[t * 128:(t + 1) * 128, :], ot[:])

    nc.compile()
    return nc


_NC_CACHE = None


def kernel(x: np.ndarray) -> np.ndarray:
    global _NC_CACHE
    assert x.shape == (ROWS, COLS) and x.dtype == np.float32
    if _NC_CACHE is None:
        _NC_CACHE = _build_module()
    nc = _NC_CACHE

    hmat = _hadamard_matrix()
    ident = np.eye(G, dtype=np.float32)
    in_maps = [
        {
            "x": np.ascontiguousarray(x[c * R_CORE:(c + 1) * R_CORE]),
            "hmat": hmat,
            "ident": ident,
        }
        for c in range(N_CORES)
    ]
    res = run_bass_kernel_spmd(nc, in_maps, core_ids=list(range(N_CORES)))
    return np.concatenate([r["out"] for r in res.results], axis=0)


# revision 3
# speedup vs baseline: 1.1782x; 1.1782x over previous
"""Hadamard transform kernel for Trainium2 (8 NeuronCores, SPMD).

Problem: x (8192, 4096) fp32; apply a 128-point Hadamard transform to each
contiguous 128-element group of every row.  Equivalent to
    out = (x.reshape(-1, 128) @ M).reshape(8192, 4096)
where M is the 128x128 butterfly matrix (symmetric, entries +/- 2^-3.5).

Strategy per core (rows sharded 8 ways -> 1024 rows/core):
  - DMA a 128-row tile [128, 4096] to SBUF (rows on partitions).
  - For each 128-col group: PE-transpose the 128x128 block into PSUM
    (contraction dim must live on partitions), copy PSUM->SBUF,
    then matmul(lhsT=block^T, rhs=M) -> PSUM gives the transformed block
    back in natural orientation; copy PSUM->SBUF and DMA the tile out.
"""

import math

import numpy as np

import concourse.bass as bass
import concourse.tile as tile
from concourse import bacc, mybir
from concourse.bass import ts
from concourse.bass_utils import run_bass_kernel_spmd

N_CORES = 8
ROWS, COLS = 8192, 4096
R_CORE = ROWS // N_CORES  # 1024 rows per core
G = 128                   # hadamard group size
NG = COLS // G            # 32 groups per row
NT = R_CORE // 128        # 8 row-tiles per core
QUADS = NG // 4           # 4 groups (one PSUM bank) per quad


def _hadamard_matrix() -> np.ndarray:
    """M = butterfly(I_128): out_row = x_row @ M (M symmetric)."""
    x = np.eye(G, dtype=np.float64)[..., None]
    for _ in range(int(math.log2(G))):
        top = x[..., ::2, :] + x[..., 1::2, :]
        bot = x[..., ::2, :] - x[..., 1::2, :]
        x = np.concatenate((top, bot), axis=-1) * (0.5 ** 0.5)
    return np.ascontiguousarray(x.squeeze(-2).astype(np.float32))


def _build_module():
    nc = bacc.Bacc("TRN2", target_bir_lowering=False, debug=False)
    f32 = mybir.dt.float32
    x_d = nc.dram_tensor("x", [R_CORE, COLS], f32, kind="ExternalInput")
    h_d = nc.dram_tensor("hmat", [G, G], f32, kind="ExternalInput")
    i_d = nc.dram_tensor("ident", [G, G], f32, kind="ExternalInput")
    o_d = nc.dram_tensor("out", [R_CORE, COLS], f32, kind="ExternalOutput")

    HC = COLS // 2  # half-tile columns (1 MB per DMA chunk)
    with tile.TileContext(nc) as tc:
        with (
            tc.tile_pool(name="const", bufs=1) as cpool,
            tc.tile_pool(name="xin", bufs=4) as xpool,
            tc.tile_pool(name="tt", bufs=6) as tpool,
            tc.tile_pool(name="outb", bufs=4) as opool,
            tc.tile_pool(name="pst", bufs=4, space=bass.MemorySpace.PSUM) as pst,
            tc.tile_pool(name="psm", bufs=4, space=bass.MemorySpace.PSUM) as psm,
        ):
            hm = cpool.tile([G, G], f32)
            idm = cpool.tile([G, G], f32)
            nc.sync.dma_start(hm[:], h_d[:])
            nc.sync.dma_start(idm[:], i_d[:])

            # loop over half-tiles: 128 rows x 2048 cols (1 MB) each.
            # input DMAs on the Sync HWDGE ring, output DMAs on the
            # Scalar HWDGE ring so loads are not queued behind stores.
            for t in range(NT):
                for h in range(2):
                    c0 = h * HC
                    xt = xpool.tile([128, HC], f32)
                    nc.sync.dma_start(
                        xt[:], x_d[t * 128:(t + 1) * 128, c0:c0 + HC]
                    )
                    ot = opool.tile([128, HC], f32)
                    for q in range(HC // 512):
                        pt = pst.tile([128, 512], f32)
                        for j in range(4):
                            g = q * 4 + j
                            nc.tensor.transpose(
                                pt[:, ts(j, G)], xt[:, ts(g, G)], idm[:]
                            )
                        tt = tpool.tile([128, 512], f32)
                        nc.vector.tensor_copy(tt[:], pt[:])
                        pm = psm.tile([128, 512], f32)
                        for j in range(4):
                            nc.tensor.matmul(
                                pm[:, ts(j, G)], tt[:, ts(j, G)], hm[:]
                            )
                        nc.scalar.copy(ot[:, ts(q, 512)], pm[:])
                    nc.scalar.dma_start(
                        o_d[t * 128:(t + 1) * 128, c0:c0 + HC], ot[:]
                    )

    nc.compile()
    return nc


_NC_CACHE = None


def kernel(x: np.ndarray) -> np.ndarray:
    global _NC_CACHE
    assert x.shape == (ROWS, COLS) and x.dtype == np.float32
    if _NC_CACHE is None:
        _NC_CACHE = _build_module()
    nc = _NC_CACHE

    hmat = _hadamard_matrix()
    ident = np.eye(G, dtype=np.float32)
    in_maps = [
        {
            "x": np.ascontiguousarray(x[c * R_CORE:(c + 1) * R_CORE]),
            "hmat": hmat,
            "ident": ident,
        }
        for c in range(N_CORES)
    ]
    res = run_bass_kernel_spmd(nc, in_maps, core_ids=list(range(N_CORES)))
    return np.concatenate([r["out"] for r in res.results], axis=0)


# revision 4
# speedup vs baseline: 1.1788x; 1.0005x over previous
"""Hadamard transform kernel for Trainium2 (8 NeuronCores, SPMD).

Problem: x (8192, 4096) fp32; apply a 128-point Hadamard transform to each
contiguous 128-element group of every row.  Equivalent to
    out = (x.reshape(-1, 128) @ M).reshape(8192, 4096)
where M is the 128x128 butterfly matrix (symmetric, entries +/- 2^-3.5).

Strategy per core (rows sharded 8 ways -> 1024 rows/core):
  - DMA a 128-row tile [128, 4096] to SBUF (rows on partitions).
  - For each 128-col group: PE-transpose the 128x128 block into PSUM
    (contraction dim must live on partitions), copy PSUM->SBUF,
    then matmul(lhsT=block^T, rhs=M) -> PSUM gives the transformed block
    back in natural orientation; copy PSUM->SBUF and DMA the tile out.
"""

import math

import numpy as np

import concourse.bass as bass
import concourse.tile as tile
from concourse import bacc, mybir
from concourse.bass import ts
from concourse.bass_utils import run_bass_kernel_spmd

N_CORES = 8
ROWS, COLS = 8192, 4096
R_CORE = ROWS // N_CORES  # 1024 rows per core
G = 128                   # hadamard group size
NG = COLS // G            # 32 groups per row
NT = R_CORE // 128        # 8 row-tiles per core
QUADS = NG // 4           # 4 groups (one PSUM bank) per quad


def _hadamard_matrix() -> np.ndarray:
    """M = butterfly(I_128): out_row = x_row @ M (M symmetric)."""
    x = np.eye(G, dtype=np.float64)[..., None]
    for _ in range(int(math.log2(G))):
        top = x[..., ::2, :] + x[..., 1::2, :]
        bot = x[..., ::2, :] - x[..., 1::2, :]
        x = np.concatenate((top, bot), axis=-1) * (0.5 ** 0.5)
    return np.ascontiguousarray(x.squeeze(-2).astype(np.float32))


def _build_module():
    nc = bacc.Bacc("TRN2", target_bir_lowering=False, debug=False)
    f32 = mybir.dt.float32
    x_d = nc.dram_tensor("x", [R_CORE, COLS], f32, kind="ExternalInput")
    h_d = nc.dram_tensor("hmat", [G, G], f32, kind="ExternalInput")
    i_d = nc.dram_tensor("ident", [G, G], f32, kind="ExternalInput")
    o_d = nc.dram_tensor("out", [R_CORE, COLS], f32, kind="ExternalOutput")

    CC = 1024  # chunk columns (512 KB per DMA chunk)
    with tile.TileContext(nc) as tc:
        with (
            tc.tile_pool(name="const", bufs=1) as cpool,
            tc.tile_pool(name="xin", bufs=6) as xpool,
            tc.tile_pool(name="tt", bufs=6) as tpool,
            tc.tile_pool(name="outb", bufs=6) as opool,
            tc.tile_pool(name="pst", bufs=4, space=bass.MemorySpace.PSUM) as pst,
            tc.tile_pool(name="psm", bufs=4, space=bass.MemorySpace.PSUM) as psm,
        ):
            hm = cpool.tile([G, G], f32)
            idm = cpool.tile([G, G], f32)
            # consts on the Scalar ring so the first x chunk leads the
            # Sync ring.
            nc.scalar.dma_start(hm[:], h_d[:])
            nc.scalar.dma_start(idm[:], i_d[:])

            # loop over chunks: 128 rows x 1024 cols (512 KB) each.
            # input DMAs on the Sync HWDGE ring, output DMAs on the
            # Scalar HWDGE ring so loads are not queued behind stores.
            for t in range(NT):
                for h in range(COLS // CC):
                    c0 = h * CC
                    xt = xpool.tile([128, CC], f32)
                    nc.sync.dma_start(
                        xt[:], x_d[t * 128:(t + 1) * 128, c0:c0 + CC]
                    )
                    ot = opool.tile([128, CC], f32)
                    for q in range(CC // 512):
                        pt = pst.tile([128, 512], f32)
                        for j in range(4):
                            g = q * 4 + j
                            nc.tensor.transpose(
                                pt[:, ts(j, G)], xt[:, ts(g, G)], idm[:]
                            )
                        tt = tpool.tile([128, 512], f32)
                        nc.vector.tensor_copy(tt[:], pt[:])
                        pm = psm.tile([128, 512], f32)
                        for j in range(4):
                            nc.tensor.matmul(
                                pm[:, ts(j, G)], tt[:, ts(j, G)], hm[:]
                            )
                        nc.scalar.copy(ot[:, ts(q, 512)], pm[:])
                    nc.scalar.dma_start(
                        o_d[t * 128:(t + 1) * 128, c0:c0 + CC], ot[:]
                    )

    nc.compile()
    return nc


_NC_CACHE = None


def kernel(x: np.ndarray) -> np.ndarray:
    global _NC_CACHE
    assert x.shape == (ROWS, COLS) and x.dtype == np.float32
    if _NC_CACHE is None:
        _NC_CACHE = _build_module()
    nc = _NC_CACHE

    hmat = _hadamard_matrix()
    ident = np.eye(G, dtype=np.float32)
    in_maps = [
        {
            "x": np.ascontiguousarray(x[c * R_CORE:(c + 1) * R_CORE]),
            "hmat": hmat,
            "ident": ident,
        }
        for c in range(N_CORES)
    ]
    res = run_bass_kernel_spmd(nc, in_maps, core_ids=list(range(N_CORES)))
    return np.concatenate([r["out"] for r in res.results], axis=0)


# revision 5
# speedup vs baseline: 1.2128x; 1.0288x over previous
"""Hadamard transform kernel for Trainium2 (8 NeuronCores, SPMD).

Problem: x (8192, 4096) fp32; apply a 128-point Hadamard transform to each
contiguous 128-element group of every row.  Equivalent to
    out = (x.reshape(-1, 128) @ M).reshape(8192, 4096)
where M is the 128x128 butterfly matrix (symmetric, entries +/- 2^-3.5).

Strategy per core (rows sharded 8 ways -> 1024 rows/core):
  - DMA a 128-row tile [128, 4096] to SBUF (rows on partitions).
  - For each 128-col group: PE-transpose the 128x128 block into PSUM
    (contraction dim must live on partitions), copy PSUM->SBUF,
    then matmul(lhsT=block^T, rhs=M) -> PSUM gives the transformed block
    back in natural orientation; copy PSUM->SBUF and DMA the tile out.
"""

import math

import numpy as np

import concourse.bass as bass
import concourse.tile as tile
from concourse import bacc, mybir
from concourse.bass import ts
from concourse.bass_utils import run_bass_kernel_spmd

N_CORES = 8
ROWS, COLS = 8192, 4096
R_CORE = ROWS // N_CORES  # 1024 rows per core
G = 128                   # hadamard group size
NG = COLS // G            # 32 groups per row
NT = R_CORE // 128        # 8 row-tiles per core
QUADS = NG // 4           # 4 groups (one PSUM bank) per quad


def _hadamard_matrix() -> np.ndarray:
    """M = butterfly(I_128): out_row = x_row @ M (M symmetric)."""
    x = np.eye(G, dtype=np.float64)[..., None]
    for _ in range(int(math.log2(G))):
        top = x[..., ::2, :] + x[..., 1::2, :]
        bot = x[..., ::2, :] - x[..., 1::2, :]
        x = np.concatenate((top, bot), axis=-1) * (0.5 ** 0.5)
    return np.ascontiguousarray(x.squeeze(-2).astype(np.float32))


def _build_module():
    nc = bacc.Bacc("TRN2", target_bir_lowering=False, debug=False)
    f32 = mybir.dt.float32
    x_d = nc.dram_tensor("x", [R_CORE, COLS], f32, kind="ExternalInput")
    h_d = nc.dram_tensor("hmat", [G, G], f32, kind="ExternalInput")
    i_d = nc.dram_tensor("ident", [G, G], f32, kind="ExternalInput")
    o_d = nc.dram_tensor("out", [R_CORE, COLS], f32, kind="ExternalOutput")

    with tile.TileContext(nc) as tc:
        with (
            tc.tile_pool(name="const", bufs=1) as cpool,
            tc.tile_pool(name="xin", bufs=6) as xpool,
            tc.tile_pool(name="tt", bufs=6) as tpool,
            tc.tile_pool(name="outb", bufs=6) as opool,
            tc.tile_pool(name="pst", bufs=4, space=bass.MemorySpace.PSUM) as pst,
            tc.tile_pool(name="psm", bufs=3, space=bass.MemorySpace.PSUM) as psm,
            tc.tile_pool(name="warm", bufs=1, space=bass.MemorySpace.PSUM) as wps,
        ):
            # PE warmup: dummy transposes with no data deps so the PE's
            # HAM clock-gate opens during the initial DMA wait.
            wsb = cpool.tile([G, G], f32)
            nc.gpsimd.memset(wsb[:], 1.0)
            wp = wps.tile([G, G], f32)
            for _ in range(20):
                nc.tensor.transpose(wp[:], wsb[:], wsb[:])

            hm = cpool.tile([G, G], f32)
            idm = cpool.tile([G, G], f32)
            nc.sync.dma_start(hm[:], h_d[:])
            nc.sync.dma_start(idm[:], i_d[:])

            # chunk the 128-row tiles along columns; small leading /
            # trailing chunks shorten pipeline fill and drain.  input
            # DMAs ride the Sync HWDGE ring, output DMAs the Scalar
            # ring so loads are not queued behind stores.
            for t in range(NT):
                if t == 0:
                    splits = [1024, 2048, 1024]
                elif t == NT - 1:
                    splits = [1024, 2048, 512, 512]
                else:
                    splits = [2048, 2048]
                c0 = 0
                for cc in splits:
                    xt = xpool.tile([128, cc], f32, tag="xt")
                    nc.sync.dma_start(
                        xt[:], x_d[t * 128:(t + 1) * 128, c0:c0 + cc]
                    )
                    ot = opool.tile([128, cc], f32, tag="ot")
                    for q in range(cc // 512):
                        pt = pst.tile([128, 512], f32)
                        for j in range(4):
                            g = q * 4 + j
                            nc.tensor.transpose(
                                pt[:, ts(j, G)], xt[:, ts(g, G)], idm[:]
                            )
                        tt = tpool.tile([128, 512], f32)
                        nc.vector.tensor_copy(tt[:], pt[:])
                        pm = psm.tile([128, 512], f32)
                        for j in range(4):
                            nc.tensor.matmul(
                                pm[:, ts(j, G)], tt[:, ts(j, G)], hm[:]
                            )
                        nc.scalar.copy(ot[:, ts(q, 512)], pm[:])
                    nc.scalar.dma_start(
                        o_d[t * 128:(t + 1) * 128, c0:c0 + cc], ot[:]
                    )
                    c0 += cc

    nc.compile()
    return nc


_NC_CACHE = None


def kernel(x: np.ndarray) -> np.ndarray:
    global _NC_CACHE
    assert x.shape == (ROWS, COLS) and x.dtype == np.float32
    if _NC_CACHE is None:
        _NC_CACHE = _build_module()
    nc = _NC_CACHE

    hmat = _hadamard_matrix()
    ident = np.eye(G, dtype=np.float32)
    in_maps = [
        {
            "x": np.ascontiguousarray(x[c * R_CORE:(c + 1) * R_CORE]),
            "hmat": hmat,
            "ident": ident,
        }
        for c in range(N_CORES)
    ]
    res = run_bass_kernel_spmd(nc, in_maps, core_ids=list(range(N_CORES)))
    return np.concatenate([r["out"] for r in res.results], axis=0)
